# revision 34
# baseline (speedup 1.0000x reference)
import sys

import numpy as np

for _p in ("/opt/trn_rl_repo",):
    if _p not in sys.path:
        sys.path.insert(0, _p)

B = 4096
M = 8192
EMB = 64
K = 4
TAU = 0.3
NCORES = 8
BLOC = B // NCORES  # 512 batch rows per core
P = 128             # batch rows per tile
NBT = BLOC // P     # 4 tiles per core
CM = 4096           # anchors per m-chunk
NCH = M // CM       # 2 chunks
SLOT = 16           # anchors per top-k slot
NSLOT = M // SLOT   # 512 slots per row
NSC = CM // SLOT    # 256 slots per chunk
NRESC = 5           # top slots rescanned (>=4 needed; 1 safety for bf16 ties)
NCAND = NRESC * SLOT  # 96 rescan candidates
HC = CM // 2        # anchors per DMA/ACT half-chunk (16KB/partition loads)

_CACHE = {}


def _build(debug=False, variant=""):
    from contextlib import ExitStack

    import concourse.bacc as bacc
    import concourse.bass as bass
    import concourse.mybir as mybir
    import concourse.tile as tile
    from concourse.masks import make_identity

    f32 = mybir.dt.float32
    bf16 = mybir.dt.bfloat16
    u32 = mybir.dt.uint32
    AF = mybir.ActivationFunctionType
    OP = mybir.AluOpType
    AX = mybir.AxisListType

    nc = bacc.Bacc()
    nodes_h = nc.declare_dram_parameter("nodes", [BLOC, 2, 2], f32, isOutput=False)
    ancS_h = nc.declare_dram_parameter("ancS", [BLOC, M, 2], f32, isOutput=False)
    ancL_h = nc.declare_dram_parameter("ancL", [BLOC, M, 2], f32, isOutput=False)
    W1_h = nc.declare_dram_parameter("W1", [EMB, 2], f32, isOutput=False)
    b1_h = nc.declare_dram_parameter("b1", [EMB], f32, isOutput=False)
    W2_h = nc.declare_dram_parameter("W2", [EMB, EMB], f32, isOutput=False)
    b2_h = nc.declare_dram_parameter("b2", [EMB], f32, isOutput=False)
    out_h = nc.declare_dram_parameter("out", [BLOC, 2 * EMB], f32, isOutput=True)
    if debug:
        dbgf_h = nc.declare_dram_parameter("dbgf", [BLOC, 2, 32], f32, isOutput=True)
        dbgi_h = nc.declare_dram_parameter("dbgi", [BLOC, 2, 16], u32, isOutput=True)

    with ExitStack() as ctx:
        tc = ctx.enter_context(tile.TileContext(nc))
        const = ctx.enter_context(tc.tile_pool(name="const", bufs=1))
        a_pool = ctx.enter_context(tc.tile_pool(name="a", bufs=8))
        sq_pool = ctx.enter_context(tc.tile_pool(name="sq", bufs=2))
        d2_pool = ctx.enter_context(tc.tile_pool(name="d2", bufs=1))
        fp1 = ctx.enter_context(tc.tile_pool(name="fp1", bufs=1))
        fp2 = ctx.enter_context(tc.tile_pool(name="fp2", bufs=2))
        slot_pool = ctx.enter_context(tc.tile_pool(name="slot", bufs=2))
        cand_pool = ctx.enter_context(tc.tile_pool(name="cand", bufs=4))
        sel_pool = ctx.enter_context(tc.tile_pool(name="sel", bufs=4))
        cwork = ctx.enter_context(tc.tile_pool(name="cwork", bufs=2))
        cw2 = ctx.enter_context(tc.tile_pool(name="cw2", bufs=1))
        small = ctx.enter_context(tc.tile_pool(name="small", bufs=2))
        mlp = ctx.enter_context(tc.tile_pool(name="mlp", bufs=1))
        psum_tp = ctx.enter_context(tc.tile_pool(name="psum_tp", bufs=1, space="PSUM"))
        psum_mm = ctx.enter_context(tc.tile_pool(name="psum_mm", bufs=1, space="PSUM"))
        psum_h2 = ctx.enter_context(tc.tile_pool(name="psum_h2", bufs=1, space="PSUM"))

        # nodes first, on gpsimd: negn gates every front ACT, so it must not
        # queue behind other const traffic
        nodes_all = const.tile([P, 4 * NBT], f32)
        nc.gpsimd.dma_start(
            out=nodes_all[:].rearrange("p (t x) -> p t x", x=4),
            in_=nodes_h[:].rearrange("(t p) a c -> p t (a c)", p=P),
        )
        negn_all = const.tile([P, 4 * NBT], f32)
        nc.gpsimd.tensor_scalar(
            out=negn_all[:], in0=nodes_all[:], scalar1=-1.0, scalar2=None, op0=OP.mult
        )

        ident = const.tile([P, P], f32)
        make_identity(nc, ident[:])

        # Warm-up Gelu: anchors the ACT table chooser on gelu_and_others
        # (gelu/square/tanh/copy) so the kernel needs a single table load.
        dummy = const.tile([1, 1], f32)
        nc.vector.memset(dummy[:], 0.0)
        nc.scalar.activation(dummy[:], dummy[:], AF.Gelu, bias=0.0, scale=1.0)

        # W1/W2 loaded straight (contiguous rows) and transposed on the tensor
        # engine — a transposing DMA of W2 costs ~35us of descriptor work
        w1sb = const.tile([EMB, 2], f32)
        nc.scalar.dma_start(out=w1sb[:], in_=W1_h[:])
        w2sb = const.tile([EMB, EMB], f32)
        nc.scalar.dma_start(out=w2sb[:], in_=W2_h[:])
        b1c = const.tile([EMB, 1], f32)
        nc.scalar.dma_start(out=b1c[:], in_=b1_h[:].rearrange("(e u) -> e u", u=1))
        b2c = const.tile([EMB, 1], f32)
        nc.scalar.dma_start(out=b2c[:], in_=b2_h[:].rearrange("(e u) -> e u", u=1))
        w1ps = psum_mm.tile([2, EMB], f32, tag="hp")
        nc.tensor.transpose(out=w1ps[:], in_=w1sb[:], identity=ident[0:EMB, 0:EMB])
        w1t = const.tile([2, EMB], f32)  # w1t[c, e] = W1[e, c]
        nc.vector.tensor_copy(w1t[:], w1ps[:])
        w2ps = psum_mm.tile([EMB, EMB], f32, tag="h2p")
        nc.tensor.transpose(out=w2ps[:], in_=w2sb[:], identity=ident[0:EMB, 0:EMB])
        w2t = const.tile([EMB, EMB], f32)  # w2t[e, f] = W2[f, e]
        nc.vector.tensor_copy(w2t[:], w2ps[:])

        # block-diag MLP weights: one matmul handles a pair of neighbors.
        # w1blk[(k c), (k' e)] = W1[e, c] * delta(k, k'); the same tile serves
        # pairs (0,1) and (2,3) since the blocks repeat.
        w1blk = const.tile([4, 2 * EMB], f32)
        nc.vector.memset(w1blk[:], 0.0)
        nc.vector.tensor_copy(w1blk[0:2, 0:EMB], w1t[:])
        # compute engines cannot address a partition base of 2; SBUF->SBUF DMA
        # has no such restriction
        nc.gpsimd.dma_start(out=w1blk[2:4, EMB:2 * EMB], in_=w1t[:])
        w2blk = const.tile([2 * EMB, 2 * EMB], f32)
        nc.vector.memset(w2blk[:], 0.0)
        nc.vector.tensor_copy(w2blk[0:EMB, 0:EMB], w2t[:])
        nc.vector.tensor_copy(w2blk[EMB:2 * EMB, EMB:2 * EMB], w2t[:])
        b1blk = const.tile([2 * EMB, 1], f32)
        nc.vector.tensor_copy(b1blk[0:EMB, :], b1c[:])
        nc.vector.tensor_copy(b1blk[EMB:2 * EMB, :], b1c[:])
        b2blk = const.tile([2 * EMB, 1], f32)
        nc.vector.tensor_copy(b2blk[0:EMB, :], b2c[:])
        nc.vector.tensor_copy(b2blk[EMB:2 * EMB, :], b2c[:])

        # iota2f[p, j] = j // 2 as f32: [0,0,1,1,...] — onehot domain over the
        # interleaved (m c) candidate layout
        iotau = cw2.tile([P, 2 * NCAND], u32, tag="oh")
        nc.gpsimd.iota(iotau[:], pattern=[[1, 2 * NCAND]], base=0, channel_multiplier=0)
        iota2u = cw2.tile([P, 2 * NCAND], u32, tag="prod")
        nc.vector.tensor_scalar(
            out=iota2u[:], in0=iotau[:], scalar1=1, scalar2=None,
            op0=OP.logical_shift_right,
        )
        iota2f = const.tile([P, 2 * NCAND], f32)
        nc.vector.tensor_copy(iota2f[:], iota2u[:])

        # rowbase_all[p, t] = (t*P + p) * NSLOT (offsets into the (b s) axis);
        # iota steps must fit int16, so build t*P + p then shift
        rowtmp = const.tile([P, NBT], u32)
        nc.gpsimd.iota(rowtmp[:], pattern=[[P, NBT]], base=0, channel_multiplier=1)
        rowbase_all = const.tile([P, NBT], u32)
        nc.vector.tensor_scalar(
            out=rowbase_all[:], in0=rowtmp[:], scalar1=9, scalar2=None,
            op0=OP.logical_shift_left,
        )

        # flat view of anchors for the slot rescan: row r = b*NSLOT + s holds
        # the 16 (x, y) pairs of slot s of batch-row b (32 f32 = 128B)
        ancS_slots = ancS_h[:].rearrange("b (s j) c -> (b s) (j c)", j=SLOT)
        ancL_slots = ancL_h[:].rearrange("b (s j) c -> (b s) (j c)", j=SLOT)

        def front(bt, br):
            """distance sweep + slot top-k + rescan gather issue"""
            rows = slice(bt * P, (bt + 1) * P)
            anc_h = ancS_h if br == 0 else ancL_h
            anc_slots = ancS_slots if br == 0 else ancL_slots
            nbx = negn_all[:, 4 * bt + 2 * br:4 * bt + 2 * br + 1]
            nby = negn_all[:, 4 * bt + 2 * br + 1:4 * bt + 2 * br + 2]

            nslot = slot_pool.tile([P, NSLOT], f32)  # -min(d2) per slot
            f2_c0 = None
            for chk in range(NCH):
                # loads, squares and pair-adds run at half-chunk
                # granularity: 8 ring slots of 16KB/partition keep the sync
                # HWDGE ring far enough ahead that ACTs never wait on DMA
                d2c = d2_pool.tile([P, CM], bf16)
                for h in range(2):
                    a_t = a_pool.tile([P, 2 * HC], f32)
                    lo = chk * CM + h * HC
                    nc.sync.dma_start(
                        out=a_t[:],
                        in_=anc_h[rows, lo:lo + HC, :].rearrange("p m c -> p (m c)"),
                    )
                    av = a_t[:].rearrange("p (m c) -> p m c", c=2)
                    u2 = sq_pool.tile([P, HC], bf16)
                    v2 = sq_pool.tile([P, HC], bf16)
                    nc.scalar.activation(
                        u2[:], av[:, :, 0], AF.Square, bias=nbx, scale=1.0
                    )
                    nc.scalar.activation(
                        v2[:], av[:, :, 1], AF.Square, bias=nby, scale=1.0
                    )
                    nc.vector.tensor_tensor(
                        out=d2c[:, h * HC:(h + 1) * HC],
                        in0=u2[:], in1=v2[:], op=OP.add,
                    )
                f1 = fp1.tile([P, CM // 2], bf16)
                dv = d2c[:].rearrange("p (s two j) -> p s two j", two=2, j=8)
                nc.vector.tensor_tensor(
                    out=f1[:].rearrange("p (s j) -> p s j", j=8),
                    in0=dv[:, :, 0, :], in1=dv[:, :, 1, :], op=OP.min,
                )
                f2 = fp2.tile([P, CM // 4], bf16)
                fv1 = f1[:].rearrange("p (s two j) -> p s two j", two=2, j=4)
                nc.vector.tensor_tensor(
                    out=f2[:].rearrange("p (s j) -> p s j", j=4),
                    in0=fv1[:, :, 0, :], in1=fv1[:, :, 1, :], op=OP.min,
                )
                nc.vector.tensor_reduce(
                    out=nslot[:, chk * NSC:(chk + 1) * NSC],
                    in_=f2[:].rearrange("p (s j) -> p s j", j=4),
                    axis=AX.X, op=OP.min, negate=True,
                )
                if chk == 0:
                    f2_c0 = f2

            svals8 = sel_pool.tile([P, 8], f32)
            nc.vector.max(out=svals8[:], in_=nslot[:])
            sidx8 = sel_pool.tile([P, 8], u32)
            nc.vector.max_index(out=sidx8[:], in_max=svals8[:], in_values=nslot[:])

            # offsets into the (b s) axis of the slot-flattened anchors
            soff = sel_pool.tile([P, 8], u32)
            nc.vector.tensor_tensor(
                out=soff[:], in0=sidx8[:],
                in1=rowbase_all[:, bt:bt + 1].to_broadcast([P, 8]), op=OP.add,
            )

            # rescan: pull the top slots' raw coords (16 pairs each).
            # HW DGE honors one offset per partition per indirect DMA, so one
            # DMA per slot rank.
            cand = cand_pool.tile([P, 2 * NCAND], f32)
            for r in range(NRESC):
                nc.gpsimd.indirect_dma_start(
                    out=cand[:, 32 * r:32 * (r + 1)],
                    out_offset=None,
                    in_=anc_slots,
                    in_offset=bass.IndirectOffsetOnAxis(
                        ap=soff[:, r:r + 1], axis=0
                    ),
                )
            return {"cand": cand, "nbx": nbx, "nby": nby, "bt": bt, "br": br,
                    "svals8": svals8, "sidx8": sidx8, "nslot": nslot,
                    "f2c0": f2_c0}

        def tail(st, gate=None):
            """exact rescan, top-4 select, coord extraction, softmax, MLP"""
            bt, br, cand = st["bt"], st["br"], st["cand"]
            rows = slice(bt * P, (bt + 1) * P)
            cv = cand[:].rearrange("p (m c) -> p m c", c=2)

            # The Tile scheduler orders by its own cost model, which assumes
            # gather DMAs land instantly — it would place these ACTs right
            # after this unit's gathers and stall scalar ~10us per unit.
            # Routing the bias through an op that also reads a LATER front's
            # nslot forces the rescan 2+ units behind its gathers.
            nbx, nby = st["nbx"], st["nby"]
            if gate is not None:
                bt_, br_ = st["bt"], st["br"]
                gat = small.tile([P, 2], f32, tag="gat")
                nc.vector.scalar_tensor_tensor(
                    out=gat[:], in0=gate[:, 0:1].to_broadcast([P, 2]), scalar=0.0,
                    in1=negn_all[:, 4 * bt_ + 2 * br_:4 * bt_ + 2 * br_ + 2],
                    op0=OP.mult, op1=OP.add,
                )
                nbx, nby = gat[:, 0:1], gat[:, 1:2]

            cu2 = cwork.tile([P, NCAND], f32)
            cv2 = cwork.tile([P, NCAND], f32)
            nc.scalar.activation(
                cu2[:], cv[:, :, 0], AF.Square, bias=nbx, scale=1.0
            )
            nc.scalar.activation(
                cv2[:], cv[:, :, 1], AF.Square, bias=nby, scale=1.0
            )
            ncd = cwork.tile([P, NCAND], f32)  # exact -d2 of candidates
            nc.vector.scalar_tensor_tensor(
                out=ncd[:], in0=cu2[:], scalar=-1.0, in1=cv2[:],
                op0=OP.mult, op1=OP.subtract,
            )

            cvals8 = small.tile([P, 8], f32)
            nc.vector.max(out=cvals8[:], in_=ncd[:])
            cpos8 = small.tile([P, 8], u32)
            nc.vector.max_index(out=cpos8[:], in_max=cvals8[:], in_values=ncd[:])
            cposf = small.tile([P, K], f32)
            nc.vector.tensor_copy(cposf[:], cpos8[:, 0:K])

            # extract the top-4 coords straight from the interleaved cand in
            # three wide ops: onehot(k x cand) -> mask -> strided pair reduce
            oh = cw2.tile([P, K * 2 * NCAND], bf16, tag="oh")
            ohv = oh[:].rearrange("p (k j) -> p k j", k=K)
            for k in range(K):
                nc.gpsimd.tensor_scalar(
                    out=ohv[:, k, :], in0=iota2f[:], scalar1=cposf[:, k:k + 1],
                    scalar2=None, op0=OP.is_equal,
                )
            prod = cw2.tile([P, K * 2 * NCAND], f32, tag="prod")
            nc.gpsimd.tensor_tensor(
                out=prod[:].rearrange("p (k j) -> p k j", k=K),
                in0=ohv,
                in1=cand[:].rearrange("p (u j) -> p u j", u=1).to_broadcast(
                    [P, K, 2 * NCAND]
                ),
                op=OP.mult,
            )
            # comb[p, 2k + c] = sum_m prod[p, k, m, c]
            comb = small.tile([P, 2 * K], f32)
            nc.vector.tensor_reduce(
                out=comb[:].rearrange("p (k c) -> p k c", c=2),
                in_=prod[:].rearrange("p (k m c) -> p k c m", c=2, m=NCAND),
                axis=AX.X, op=OP.add,
            )

            # w = softmax(d2_topk / tau); cvals8[:, :K] = -d2 (descending),
            # so the softmax argmax is at column K-1.
            nrmaxh = small.tile([P, 1], f32)
            nc.gpsimd.tensor_scalar(
                out=nrmaxh[:], in0=cvals8[:, K - 1:K], scalar1=float(0.5 / TAU),
                scalar2=None, op0=OP.mult,
            )
            # exp(x) = 2/(1 - tanh(x/2)) - 1; keeps ACT inside the
            # gelu_and_others table set (no exp there).
            th = small.tile([P, K], f32)
            nc.scalar.activation(
                th[:], cvals8[:, 0:K], AF.Tanh,
                bias=nrmaxh[:, 0:1], scale=float(-0.5 / TAU),
            )
            denom = small.tile([P, K], f32)
            nc.gpsimd.tensor_scalar(
                out=denom[:], in0=th[:], scalar1=-1.0, scalar2=1.0,
                op0=OP.mult, op1=OP.add,
            )
            rden = small.tile([P, K], f32)
            nc.vector.reciprocal(rden[:], denom[:])
            e4 = small.tile([P, K], f32)
            nc.gpsimd.tensor_scalar(
                out=e4[:], in0=rden[:], scalar1=2.0, scalar2=-1.0,
                op0=OP.mult, op1=OP.add,
            )
            ssum = small.tile([P, 1], f32)
            nc.vector.reduce_sum(out=ssum[:], in_=e4[:], axis=AX.X)
            rinv = small.tile([P, 1], f32)
            nc.vector.reciprocal(rinv[:], ssum[:])
            wnorm = small.tile([P, K], f32)
            nc.vector.tensor_scalar(
                out=wnorm[:], in0=e4[:], scalar1=rinv[:, 0:1],
                scalar2=None, op0=OP.mult,
            )

            # comb [P, (k c)] -> combT [(k c), P] in one transpose; the
            # block-diag weights then run the MLP for a k-pair per matmul
            # each k-pair's coords transposed into its own base-0 tile
            # (partition bases must be 0/32/64 for compute-engine access)
            ct_ps_a = psum_tp.tile([4, P], f32, tag="ct_a")
            nc.tensor.transpose(out=ct_ps_a[:], in_=comb[:, 0:4], identity=ident[:])
            ct_ps_b = psum_tp.tile([4, P], f32, tag="ct_b")
            nc.tensor.transpose(out=ct_ps_b[:], in_=comb[:, 4:8], identity=ident[:])
            # both k-pairs side by side: one matmul and one gelu per layer
            combT = mlp.tile([4, 2 * P], f32)
            nc.vector.tensor_copy(combT[:, 0:P], ct_ps_a[:])
            nc.vector.tensor_copy(combT[:, P:2 * P], ct_ps_b[:])

            hp = psum_mm.tile([2 * EMB, 2 * P], f32, tag="hp")
            nc.tensor.matmul(
                out=hp[:], lhsT=w1blk[:], rhs=combT[:], start=True, stop=True
            )
            h1 = mlp.tile([2 * EMB, 2 * P], f32)
            nc.scalar.activation(
                h1[:], hp[:], AF.Gelu, bias=b1blk[:, 0:1], scale=1.0
            )
            h2p = psum_mm.tile([2 * EMB, 2 * P], f32, tag="h2p")
            nc.tensor.matmul(
                out=h2p[:], lhsT=w2blk[:], rhs=h1[:], start=True, stop=True
            )
            h2 = mlp.tile([2 * EMB, 2 * P], f32)
            nc.scalar.activation(
                h2[:], h2p[:], AF.Gelu, bias=b2blk[:, 0:1], scale=1.0
            )

            # weighted sum over neighbors, back in b-on-partitions layout
            osb = small.tile([P, EMB], f32)
            h2t_a = psum_h2.tile([P, 2 * EMB], f32, tag="h2t_a")
            nc.tensor.transpose(out=h2t_a[:], in_=h2[:, 0:P], identity=ident[:])
            h2t_b = psum_h2.tile([P, 2 * EMB], f32, tag="h2t_b")
            nc.tensor.transpose(out=h2t_b[:], in_=h2[:, P:2 * P], identity=ident[:])
            nc.vector.tensor_scalar(
                out=osb[:], in0=h2t_a[:, 0:EMB], scalar1=wnorm[:, 0:1],
                scalar2=None, op0=OP.mult,
            )
            for k in range(1, K):
                srct = h2t_a if k < 2 else h2t_b
                col = (k % 2) * EMB
                nc.vector.scalar_tensor_tensor(
                    out=osb[:], in0=srct[:, col:col + EMB],
                    scalar=wnorm[:, k:k + 1],
                    in1=osb[:], op0=OP.mult, op1=OP.add,
                )
            # out store deferred to the next tail (issued from flush_out):
            # issuing it here would park gpsimd on the osb semaphore and
            # head-of-line block the next unit's gather DMAs
            st["osb"] = osb
            if debug:
                dbf = small.tile([P, 32], f32)
                nc.vector.tensor_copy(dbf[:, 0:8], cvals8[:])
                nc.vector.tensor_copy(dbf[:, 8:16], comb[:])
                nc.vector.tensor_copy(dbf[:, 16:20], wnorm[:])
                nc.vector.tensor_copy(dbf[:, 20:28], st["svals8"][:])
                nc.gpsimd.dma_start(out=dbgf_h[rows, br, :], in_=dbf[:])
                dbi = small.tile([P, 16], u32)
                nc.vector.tensor_copy(dbi[:, 0:8], st["sidx8"][:])
                nc.vector.tensor_copy(dbi[:, 8:16], cpos8[:])
                nc.gpsimd.dma_start(out=dbgi_h[rows, br, :], in_=dbi[:])

        def flush_out(st):
            """issue the (now long-ready) out store for an already-tailed unit"""
            rows = slice(st["bt"] * P, (st["bt"] + 1) * P)
            nc.gpsimd.dma_start(
                out=out_h[rows, st["br"] * EMB:(st["br"] + 1) * EMB],
                in_=st["osb"][:],
            )

        # software pipeline, 3 units deep: tail(i) is issued after front(i+3),
        # so a unit's select->gather->rescan chain (including the ~8us gather
        # completion latency) is hidden; out stores trail one more tail behind
        pending = []
        done = []
        for bt in range(NBT):
            for br in range(2):
                pending.append(front(bt, br))
                if len(pending) > 3:
                    st = pending.pop(0)
                    tail(st, gate=pending[1]["f2c0"][:])
                    done.append(st)
                    if len(done) > 1:
                        flush_out(done.pop(0))
        while pending:
            st = pending.pop(0)
            tail(st)
            done.append(st)
        for st in done:
            flush_out(st)
    return nc


def _get_nc(debug=False, variant=""):
    key = ("nc", debug, variant)
    if key not in _CACHE:
        nc = _build(debug, variant)
        nc.finalize()  # runs the Bacc passes (event sems, reg alloc, table loads)
        _CACHE[key] = nc
    return _CACHE[key]


def _make_in_maps(inputs):
    nodes = np.asarray(inputs["nodes_2x2"], dtype=np.float32)
    ancS = np.asarray(inputs["ancS"], dtype=np.float32)
    ancL = np.asarray(inputs["ancL"], dtype=np.float32)
    W1 = np.asarray(inputs["W1"], dtype=np.float32)
    b1 = np.asarray(inputs["b1"], dtype=np.float32)
    W2 = np.asarray(inputs["W2"], dtype=np.float32)
    b2 = np.asarray(inputs["b2"], dtype=np.float32)
    in_maps = []
    for c in range(NCORES):
        sl = slice(c * BLOC, (c + 1) * BLOC)
        in_maps.append(
            {
                "nodes": np.ascontiguousarray(nodes[sl]),
                "ancS": np.ascontiguousarray(ancS[sl]),
                "ancL": np.ascontiguousarray(ancL[sl]),
                "W1": W1,
                "b1": b1,
                "W2": W2,
                "b2": b2,
            }
        )
    return in_maps


def _run(in_maps, trace=False, debug=False, variant=""):
    from concourse.bass_utils import run_bass_kernel_spmd

    nc = _get_nc(debug, variant)
    return run_bass_kernel_spmd(nc, in_maps, core_ids=list(range(NCORES)), trace=trace)


def kernel(**inputs):
    in_maps = _make_in_maps(inputs)
    res = _run(in_maps).results
    out = np.concatenate([res[c]["out"] for c in range(NCORES)], axis=0)
    return out[:, :EMB].copy(), out[:, EMB:].copy()


# revision 35
# speedup vs baseline: 1.2502x; 1.2502x over previous
import sys

import numpy as np

for _p in ("/opt/trn_rl_repo",):
    if _p not in sys.path:
        sys.path.insert(0, _p)

B = 4096
M = 8192
EMB = 64
K = 4
TAU = 0.3
NCORES = 8
BLOC = B // NCORES  # 512 batch rows per core
P = 128             # batch rows per tile
NBT = BLOC // P     # 4 tiles per core
CM = 4096           # anchors per m-chunk
NCH = M // CM       # 2 chunks
SLOT = 16           # anchors per top-k slot
NSLOT = M // SLOT   # 512 slots per row
NSC = CM // SLOT    # 256 slots per chunk
NRESC = 5           # top slots rescanned (>=4 needed; 1 safety for bf16 ties)
NCAND = NRESC * SLOT  # 96 rescan candidates
HC = CM // 2        # anchors per DMA/ACT half-chunk (16KB/partition loads)

_CACHE = {}


def _build(debug=False, variant=""):
    from contextlib import ExitStack

    import concourse.bacc as bacc
    import concourse.bass as bass
    import concourse.mybir as mybir
    import concourse.tile as tile
    from concourse.masks import make_identity

    f32 = mybir.dt.float32
    bf16 = mybir.dt.bfloat16
    u32 = mybir.dt.uint32
    AF = mybir.ActivationFunctionType
    OP = mybir.AluOpType
    AX = mybir.AxisListType

    nc = bacc.Bacc()
    nodes_h = nc.declare_dram_parameter("nodes", [BLOC, 2, 2], f32, isOutput=False)
    ancS_h = nc.declare_dram_parameter("ancS", [BLOC, M, 2], f32, isOutput=False)
    ancL_h = nc.declare_dram_parameter("ancL", [BLOC, M, 2], f32, isOutput=False)
    W1_h = nc.declare_dram_parameter("W1", [EMB, 2], f32, isOutput=False)
    b1_h = nc.declare_dram_parameter("b1", [EMB], f32, isOutput=False)
    W2_h = nc.declare_dram_parameter("W2", [EMB, EMB], f32, isOutput=False)
    b2_h = nc.declare_dram_parameter("b2", [EMB], f32, isOutput=False)
    out_h = nc.declare_dram_parameter("out", [BLOC, 2 * EMB], f32, isOutput=True)
    if debug:
        dbgf_h = nc.declare_dram_parameter("dbgf", [BLOC, 2, 32], f32, isOutput=True)
        dbgi_h = nc.declare_dram_parameter("dbgi", [BLOC, 2, 16], u32, isOutput=True)

    with ExitStack() as ctx:
        tc = ctx.enter_context(tile.TileContext(nc))
        const = ctx.enter_context(tc.tile_pool(name="const", bufs=1))
        a_pool = ctx.enter_context(tc.tile_pool(name="a", bufs=8))
        sq_pool = ctx.enter_context(tc.tile_pool(name="sq", bufs=2))
        d2_pool = ctx.enter_context(tc.tile_pool(name="d2", bufs=1))
        fp1 = ctx.enter_context(tc.tile_pool(name="fp1", bufs=1))
        fp2 = ctx.enter_context(tc.tile_pool(name="fp2", bufs=1))
        slot_pool = ctx.enter_context(tc.tile_pool(name="slot", bufs=2))
        cand_pool = ctx.enter_context(tc.tile_pool(name="cand", bufs=4))
        sel_pool = ctx.enter_context(tc.tile_pool(name="sel", bufs=4))
        cwork = ctx.enter_context(tc.tile_pool(name="cwork", bufs=2))
        cw2 = ctx.enter_context(tc.tile_pool(name="cw2", bufs=1))
        small = ctx.enter_context(tc.tile_pool(name="small", bufs=2))
        mlp = ctx.enter_context(tc.tile_pool(name="mlp", bufs=1))
        psum_tp = ctx.enter_context(tc.tile_pool(name="psum_tp", bufs=1, space="PSUM"))
        psum_mm = ctx.enter_context(tc.tile_pool(name="psum_mm", bufs=1, space="PSUM"))
        psum_h2 = ctx.enter_context(tc.tile_pool(name="psum_h2", bufs=1, space="PSUM"))

        # nodes first, on gpsimd: negn gates every front ACT, so it must not
        # queue behind other const traffic
        nodes_all = const.tile([P, 4 * NBT], f32)
        nc.gpsimd.dma_start(
            out=nodes_all[:].rearrange("p (t x) -> p t x", x=4),
            in_=nodes_h[:].rearrange("(t p) a c -> p t (a c)", p=P),
        )
        negn_all = const.tile([P, 4 * NBT], f32)
        nc.gpsimd.tensor_scalar(
            out=negn_all[:], in0=nodes_all[:], scalar1=-1.0, scalar2=None, op0=OP.mult
        )

        ident = const.tile([P, P], f32)
        make_identity(nc, ident[:])

        # Warm-up Gelu: anchors the ACT table chooser on gelu_and_others
        # (gelu/square/tanh/copy) so the kernel needs a single table load.
        dummy = const.tile([1, 1], f32)
        nc.vector.memset(dummy[:], 0.0)
        nc.scalar.activation(dummy[:], dummy[:], AF.Gelu, bias=0.0, scale=1.0)

        # W1/W2 loaded straight (contiguous rows) and transposed on the tensor
        # engine — a transposing DMA of W2 costs ~35us of descriptor work
        w1sb = const.tile([EMB, 2], f32)
        nc.scalar.dma_start(out=w1sb[:], in_=W1_h[:])
        w2sb = const.tile([EMB, EMB], f32)
        nc.scalar.dma_start(out=w2sb[:], in_=W2_h[:])
        b1c = const.tile([EMB, 1], f32)
        nc.scalar.dma_start(out=b1c[:], in_=b1_h[:].rearrange("(e u) -> e u", u=1))
        b2c = const.tile([EMB, 1], f32)
        nc.scalar.dma_start(out=b2c[:], in_=b2_h[:].rearrange("(e u) -> e u", u=1))
        w1ps = psum_mm.tile([2, EMB], f32, tag="hp")
        nc.tensor.transpose(out=w1ps[:], in_=w1sb[:], identity=ident[0:EMB, 0:EMB])
        w1t = const.tile([2, EMB], f32)  # w1t[c, e] = W1[e, c]
        nc.vector.tensor_copy(w1t[:], w1ps[:])
        w2ps = psum_mm.tile([EMB, EMB], f32, tag="h2p")
        nc.tensor.transpose(out=w2ps[:], in_=w2sb[:], identity=ident[0:EMB, 0:EMB])
        w2t = const.tile([EMB, EMB], f32)  # w2t[e, f] = W2[f, e]
        nc.vector.tensor_copy(w2t[:], w2ps[:])

        # block-diag MLP weights: one matmul handles a pair of neighbors.
        # w1blk[(k c), (k' e)] = W1[e, c] * delta(k, k'); the same tile serves
        # pairs (0,1) and (2,3) since the blocks repeat.
        w1blk = const.tile([4, 2 * EMB], f32)
        nc.vector.memset(w1blk[:], 0.0)
        nc.vector.tensor_copy(w1blk[0:2, 0:EMB], w1t[:])
        # compute engines cannot address a partition base of 2; SBUF->SBUF DMA
        # has no such restriction
        nc.gpsimd.dma_start(out=w1blk[2:4, EMB:2 * EMB], in_=w1t[:])
        w2blk = const.tile([2 * EMB, 2 * EMB], f32)
        nc.vector.memset(w2blk[:], 0.0)
        nc.vector.tensor_copy(w2blk[0:EMB, 0:EMB], w2t[:])
        nc.vector.tensor_copy(w2blk[EMB:2 * EMB, EMB:2 * EMB], w2t[:])
        b1blk = const.tile([2 * EMB, 1], f32)
        nc.vector.tensor_copy(b1blk[0:EMB, :], b1c[:])
        nc.vector.tensor_copy(b1blk[EMB:2 * EMB, :], b1c[:])
        b2blk = const.tile([2 * EMB, 1], f32)
        nc.vector.tensor_copy(b2blk[0:EMB, :], b2c[:])
        nc.vector.tensor_copy(b2blk[EMB:2 * EMB, :], b2c[:])

        # iota2f[p, j] = j // 2 as f32: [0,0,1,1,...] — onehot domain over the
        # interleaved (m c) candidate layout
        iotau = cw2.tile([P, 2 * NCAND], u32, tag="oh")
        nc.gpsimd.iota(iotau[:], pattern=[[1, 2 * NCAND]], base=0, channel_multiplier=0)
        iota2u = cw2.tile([P, 2 * NCAND], u32, tag="prod")
        nc.vector.tensor_scalar(
            out=iota2u[:], in0=iotau[:], scalar1=1, scalar2=None,
            op0=OP.logical_shift_right,
        )
        iota2f = const.tile([P, 2 * NCAND], f32)
        nc.vector.tensor_copy(iota2f[:], iota2u[:])

        # rowbase_all[p, t] = (t*P + p) * NSLOT (offsets into the (b s) axis);
        # iota steps must fit int16, so build t*P + p then shift
        rowtmp = const.tile([P, NBT], u32)
        nc.gpsimd.iota(rowtmp[:], pattern=[[P, NBT]], base=0, channel_multiplier=1)
        rowbase_all = const.tile([P, NBT], u32)
        nc.vector.tensor_scalar(
            out=rowbase_all[:], in0=rowtmp[:], scalar1=9, scalar2=None,
            op0=OP.logical_shift_left,
        )

        # flat view of anchors for the slot rescan: row r = b*NSLOT + s holds
        # the 16 (x, y) pairs of slot s of batch-row b (32 f32 = 128B)
        ancS_slots = ancS_h[:].rearrange("b (s j) c -> (b s) (j c)", j=SLOT)
        ancL_slots = ancL_h[:].rearrange("b (s j) c -> (b s) (j c)", j=SLOT)

        def front(bt, br):
            """distance sweep + slot top-k + rescan gather issue"""
            rows = slice(bt * P, (bt + 1) * P)
            anc_h = ancS_h if br == 0 else ancL_h
            anc_slots = ancS_slots if br == 0 else ancL_slots
            nbx = negn_all[:, 4 * bt + 2 * br:4 * bt + 2 * br + 1]
            nby = negn_all[:, 4 * bt + 2 * br + 1:4 * bt + 2 * br + 2]

            nslot = slot_pool.tile([P, NSLOT], f32)  # -min(d2) per slot
            f2_c0 = None
            for chk in range(NCH):
                # loads, squares and pair-adds run at half-chunk
                # granularity: 8 ring slots of 16KB/partition keep the sync
                # HWDGE ring far enough ahead that ACTs never wait on DMA
                d2c = d2_pool.tile([P, CM], bf16)
                for h in range(2):
                    a_t = a_pool.tile([P, 2 * HC], f32)
                    lo = chk * CM + h * HC
                    nc.sync.dma_start(
                        out=a_t[:],
                        in_=anc_h[rows, lo:lo + HC, :].rearrange("p m c -> p (m c)"),
                    )
                    av = a_t[:].rearrange("p (m c) -> p m c", c=2)
                    u2 = sq_pool.tile([P, HC], bf16)
                    v2 = sq_pool.tile([P, HC], bf16)
                    nc.scalar.activation(
                        u2[:], av[:, :, 0], AF.Square, bias=nbx, scale=1.0
                    )
                    nc.scalar.activation(
                        v2[:], av[:, :, 1], AF.Square, bias=nby, scale=1.0
                    )
                    nc.vector.tensor_tensor(
                        out=d2c[:, h * HC:(h + 1) * HC],
                        in0=u2[:], in1=v2[:], op=OP.add,
                    )
                f1 = fp1.tile([P, CM // 2], bf16)
                dv = d2c[:].rearrange("p (s two j) -> p s two j", two=2, j=8)
                nc.vector.tensor_tensor(
                    out=f1[:].rearrange("p (s j) -> p s j", j=8),
                    in0=dv[:, :, 0, :], in1=dv[:, :, 1, :], op=OP.min,
                )
                f2 = fp2.tile([P, CM // 4], bf16)
                fv1 = f1[:].rearrange("p (s two j) -> p s two j", two=2, j=4)
                nc.vector.tensor_tensor(
                    out=f2[:].rearrange("p (s j) -> p s j", j=4),
                    in0=fv1[:, :, 0, :], in1=fv1[:, :, 1, :], op=OP.min,
                )
                nc.vector.tensor_reduce(
                    out=nslot[:, chk * NSC:(chk + 1) * NSC],
                    in_=f2[:].rearrange("p (s j) -> p s j", j=4),
                    axis=AX.X, op=OP.min, negate=True,
                )
                if chk == 0:
                    f2_c0 = f2

            svals8 = sel_pool.tile([P, 8], f32)
            nc.vector.max(out=svals8[:], in_=nslot[:])
            sidx8 = sel_pool.tile([P, 8], u32)
            nc.vector.max_index(out=sidx8[:], in_max=svals8[:], in_values=nslot[:])

            # offsets into the (b s) axis of the slot-flattened anchors
            soff = sel_pool.tile([P, 8], u32)
            nc.vector.tensor_tensor(
                out=soff[:], in0=sidx8[:],
                in1=rowbase_all[:, bt:bt + 1].to_broadcast([P, 8]), op=OP.add,
            )

            # rescan: pull the top slots' raw coords (16 pairs each).
            # HW DGE honors one offset per partition per indirect DMA, so one
            # DMA per slot rank.
            cand = cand_pool.tile([P, 2 * NCAND], f32)
            for r in range(NRESC):
                nc.gpsimd.indirect_dma_start(
                    out=cand[:, 32 * r:32 * (r + 1)],
                    out_offset=None,
                    in_=anc_slots,
                    in_offset=bass.IndirectOffsetOnAxis(
                        ap=soff[:, r:r + 1], axis=0
                    ),
                )
            return {"cand": cand, "nbx": nbx, "nby": nby, "bt": bt, "br": br,
                    "svals8": svals8, "sidx8": sidx8, "nslot": nslot,
                    "f2c0": f2_c0}

        def tail(st, gate=None):
            """exact rescan, top-4 select, coord extraction, softmax, MLP"""
            bt, br, cand = st["bt"], st["br"], st["cand"]
            rows = slice(bt * P, (bt + 1) * P)
            cv = cand[:].rearrange("p (m c) -> p m c", c=2)

            # The Tile scheduler orders by its own cost model, which assumes
            # gather DMAs land instantly — it would place these ACTs right
            # after this unit's gathers and stall scalar ~10us per unit.
            # Routing the bias through an op that also reads a LATER front's
            # nslot forces the rescan 2+ units behind its gathers.
            nbx, nby = st["nbx"], st["nby"]
            if gate is not None:
                bt_, br_ = st["bt"], st["br"]
                gat = small.tile([P, 2], f32, tag="gat")
                nc.vector.scalar_tensor_tensor(
                    out=gat[:], in0=gate[:, 0:1].to_broadcast([P, 2]), scalar=0.0,
                    in1=negn_all[:, 4 * bt_ + 2 * br_:4 * bt_ + 2 * br_ + 2],
                    op0=OP.mult, op1=OP.add,
                )
                nbx, nby = gat[:, 0:1], gat[:, 1:2]

            cu2 = cwork.tile([P, NCAND], f32)
            cv2 = cwork.tile([P, NCAND], f32)
            nc.scalar.activation(
                cu2[:], cv[:, :, 0], AF.Square, bias=nbx, scale=1.0
            )
            nc.scalar.activation(
                cv2[:], cv[:, :, 1], AF.Square, bias=nby, scale=1.0
            )
            ncd = cwork.tile([P, NCAND], f32)  # exact -d2 of candidates
            nc.vector.scalar_tensor_tensor(
                out=ncd[:], in0=cu2[:], scalar=-1.0, in1=cv2[:],
                op0=OP.mult, op1=OP.subtract,
            )

            cvals8 = small.tile([P, 8], f32)
            nc.vector.max(out=cvals8[:], in_=ncd[:])
            cpos8 = small.tile([P, 8], u32)
            nc.vector.max_index(out=cpos8[:], in_max=cvals8[:], in_values=ncd[:])
            cposf = small.tile([P, K], f32)
            nc.vector.tensor_copy(cposf[:], cpos8[:, 0:K])

            # extract the top-4 coords straight from the interleaved cand in
            # three wide ops: onehot(k x cand) -> mask -> strided pair reduce
            oh = cw2.tile([P, K * 2 * NCAND], bf16, tag="oh")
            ohv = oh[:].rearrange("p (k j) -> p k j", k=K)
            nc.vector.tensor_tensor(
                out=ohv,
                in0=iota2f[:].rearrange("p (u j) -> p u j", u=1).to_broadcast(
                    [P, K, 2 * NCAND]
                ),
                in1=cposf[:].rearrange("p (k u) -> p k u", u=1).to_broadcast(
                    [P, K, 2 * NCAND]
                ),
                op=OP.is_equal,
            )
            prod = cw2.tile([P, K * 2 * NCAND], f32, tag="prod")
            nc.gpsimd.tensor_tensor(
                out=prod[:].rearrange("p (k j) -> p k j", k=K),
                in0=ohv,
                in1=cand[:].rearrange("p (u j) -> p u j", u=1).to_broadcast(
                    [P, K, 2 * NCAND]
                ),
                op=OP.mult,
            )
            # comb[p, 2k + c] = sum_m prod[p, k, m, c]
            comb = small.tile([P, 2 * K], f32)
            nc.vector.tensor_reduce(
                out=comb[:].rearrange("p (k c) -> p k c", c=2),
                in_=prod[:].rearrange("p (k m c) -> p k c m", c=2, m=NCAND),
                axis=AX.X, op=OP.add,
            )

            # w = softmax(d2_topk / tau); cvals8[:, :K] = -d2 (descending),
            # so the softmax argmax is at column K-1.
            nrmaxh = small.tile([P, 1], f32)
            nc.gpsimd.tensor_scalar(
                out=nrmaxh[:], in0=cvals8[:, K - 1:K], scalar1=float(0.5 / TAU),
                scalar2=None, op0=OP.mult,
            )
            # exp(x) = 2/(1 - tanh(x/2)) - 1; keeps ACT inside the
            # gelu_and_others table set (no exp there).
            th = small.tile([P, K], f32)
            nc.scalar.activation(
                th[:], cvals8[:, 0:K], AF.Tanh,
                bias=nrmaxh[:, 0:1], scale=float(-0.5 / TAU),
            )
            denom = small.tile([P, K], f32)
            nc.gpsimd.tensor_scalar(
                out=denom[:], in0=th[:], scalar1=-1.0, scalar2=1.0,
                op0=OP.mult, op1=OP.add,
            )
            rden = small.tile([P, K], f32)
            nc.vector.reciprocal(rden[:], denom[:])
            e4 = small.tile([P, K], f32)
            nc.gpsimd.tensor_scalar(
                out=e4[:], in0=rden[:], scalar1=2.0, scalar2=-1.0,
                op0=OP.mult, op1=OP.add,
            )
            ssum = small.tile([P, 1], f32)
            nc.vector.reduce_sum(out=ssum[:], in_=e4[:], axis=AX.X)
            rinv = small.tile([P, 1], f32)
            nc.vector.reciprocal(rinv[:], ssum[:])
            wnorm = small.tile([P, K], f32)
            nc.vector.tensor_scalar(
                out=wnorm[:], in0=e4[:], scalar1=rinv[:, 0:1],
                scalar2=None, op0=OP.mult,
            )

            # comb [P, (k c)] -> combT [(k c), P] in one transpose; the
            # block-diag weights then run the MLP for a k-pair per matmul
            # each k-pair's coords transposed into its own base-0 tile
            # (partition bases must be 0/32/64 for compute-engine access)
            ct_ps_a = psum_tp.tile([4, P], f32, tag="ct_a")
            nc.tensor.transpose(out=ct_ps_a[:], in_=comb[:, 0:4], identity=ident[:])
            ct_ps_b = psum_tp.tile([4, P], f32, tag="ct_b")
            nc.tensor.transpose(out=ct_ps_b[:], in_=comb[:, 4:8], identity=ident[:])
            # both k-pairs side by side: one matmul and one gelu per layer
            combT = mlp.tile([4, 2 * P], f32)
            nc.vector.tensor_copy(combT[:, 0:P], ct_ps_a[:])
            nc.vector.tensor_copy(combT[:, P:2 * P], ct_ps_b[:])

            hp = psum_mm.tile([2 * EMB, 2 * P], f32, tag="hp")
            nc.tensor.matmul(
                out=hp[:], lhsT=w1blk[:], rhs=combT[:], start=True, stop=True
            )
            h1 = mlp.tile([2 * EMB, 2 * P], f32)
            nc.scalar.activation(
                h1[:], hp[:], AF.Gelu, bias=b1blk[:, 0:1], scale=1.0
            )
            h2p = psum_mm.tile([2 * EMB, 2 * P], f32, tag="h2p")
            nc.tensor.matmul(
                out=h2p[:], lhsT=w2blk[:], rhs=h1[:], start=True, stop=True
            )
            h2 = mlp.tile([2 * EMB, 2 * P], f32)
            nc.scalar.activation(
                h2[:], h2p[:], AF.Gelu, bias=b2blk[:, 0:1], scale=1.0
            )

            # weighted sum over neighbors, back in b-on-partitions layout
            osb = small.tile([P, EMB], f32)
            h2t_a = psum_h2.tile([P, 2 * EMB], f32, tag="h2t_a")
            nc.tensor.transpose(out=h2t_a[:], in_=h2[:, 0:P], identity=ident[:])
            h2t_b = psum_h2.tile([P, 2 * EMB], f32, tag="h2t_b")
            nc.tensor.transpose(out=h2t_b[:], in_=h2[:, P:2 * P], identity=ident[:])
            nc.vector.tensor_scalar(
                out=osb[:], in0=h2t_a[:, 0:EMB], scalar1=wnorm[:, 0:1],
                scalar2=None, op0=OP.mult,
            )
            for k in range(1, K):
                srct = h2t_a if k < 2 else h2t_b
                col = (k % 2) * EMB
                nc.vector.scalar_tensor_tensor(
                    out=osb[:], in0=srct[:, col:col + EMB],
                    scalar=wnorm[:, k:k + 1],
                    in1=osb[:], op0=OP.mult, op1=OP.add,
                )
            # out store deferred to the next tail (issued from flush_out):
            # issuing it here would park gpsimd on the osb semaphore and
            # head-of-line block the next unit's gather DMAs
            st["osb"] = osb
            if debug:
                dbf = small.tile([P, 32], f32)
                nc.vector.tensor_copy(dbf[:, 0:8], cvals8[:])
                nc.vector.tensor_copy(dbf[:, 8:16], comb[:])
                nc.vector.tensor_copy(dbf[:, 16:20], wnorm[:])
                nc.vector.tensor_copy(dbf[:, 20:28], st["svals8"][:])
                nc.gpsimd.dma_start(out=dbgf_h[rows, br, :], in_=dbf[:])
                dbi = small.tile([P, 16], u32)
                nc.vector.tensor_copy(dbi[:, 0:8], st["sidx8"][:])
                nc.vector.tensor_copy(dbi[:, 8:16], cpos8[:])
                nc.gpsimd.dma_start(out=dbgi_h[rows, br, :], in_=dbi[:])

        def flush_out(st):
            """issue the (now long-ready) out store for an already-tailed unit"""
            rows = slice(st["bt"] * P, (st["bt"] + 1) * P)
            nc.gpsimd.dma_start(
                out=out_h[rows, st["br"] * EMB:(st["br"] + 1) * EMB],
                in_=st["osb"][:],
            )

        # software pipeline, 3 units deep: tail(i) is issued after front(i+3),
        # so a unit's select->gather->rescan chain (including the ~8us gather
        # completion latency) is hidden; out stores trail one more tail behind
        pending = []
        done = []
        for bt in range(NBT):
            for br in range(2):
                pending.append(front(bt, br))
                if len(pending) > 3:
                    st = pending.pop(0)
                    tail(st, gate=pending[1]["nslot"][:])
                    done.append(st)
                    if len(done) > 1:
                        flush_out(done.pop(0))
        while pending:
            st = pending.pop(0)
            tail(st)
            done.append(st)
        for st in done:
            flush_out(st)
    return nc


def _get_nc(debug=False, variant=""):
    key = ("nc", debug, variant)
    if key not in _CACHE:
        nc = _build(debug, variant)
        nc.finalize()  # runs the Bacc passes (event sems, reg alloc, table loads)
        _CACHE[key] = nc
    return _CACHE[key]


def _make_in_maps(inputs):
    nodes = np.asarray(inputs["nodes_2x2"], dtype=np.float32)
    ancS = np.asarray(inputs["ancS"], dtype=np.float32)
    ancL = np.asarray(inputs["ancL"], dtype=np.float32)
    W1 = np.asarray(inputs["W1"], dtype=np.float32)
    b1 = np.asarray(inputs["b1"], dtype=np.float32)
    W2 = np.asarray(inputs["W2"], dtype=np.float32)
    b2 = np.asarray(inputs["b2"], dtype=np.float32)
    in_maps = []
    for c in range(NCORES):
        sl = slice(c * BLOC, (c + 1) * BLOC)
        in_maps.append(
            {
                "nodes": np.ascontiguousarray(nodes[sl]),
                "ancS": np.ascontiguousarray(ancS[sl]),
                "ancL": np.ascontiguousarray(ancL[sl]),
                "W1": W1,
                "b1": b1,
                "W2": W2,
                "b2": b2,
            }
        )
    return in_maps


def _run(in_maps, trace=False, debug=False, variant=""):
    from concourse.bass_utils import run_bass_kernel_spmd

    nc = _get_nc(debug, variant)
    return run_bass_kernel_spmd(nc, in_maps, core_ids=list(range(NCORES)), trace=trace)


def kernel(**inputs):
    in_maps = _make_in_maps(inputs)
    res = _run(in_maps).results
    out = np.concatenate([res[c]["out"] for c in range(NCORES)], axis=0)
    return out[:, :EMB].copy(), out[:, EMB:].copy()


# revision 36
# speedup vs baseline: 1.3275x; 1.0619x over previous
import sys

import numpy as np

for _p in ("/opt/trn_rl_repo",):
    if _p not in sys.path:
        sys.path.insert(0, _p)

B = 4096
M = 8192
EMB = 64
K = 4
TAU = 0.3
NCORES = 8
BLOC = B // NCORES  # 512 batch rows per core
P = 128             # batch rows per tile
NBT = BLOC // P     # 4 tiles per core
CM = 4096           # anchors per m-chunk
NCH = M // CM       # 2 chunks
SLOT = 16           # anchors per top-k slot
NSLOT = M // SLOT   # 512 slots per row
NSC = CM // SLOT    # 256 slots per chunk
NRESC = 5           # top slots rescanned (>=4 needed; 1 safety for bf16 ties)
NCAND = NRESC * SLOT  # 96 rescan candidates
HC = CM // 2        # anchors per DMA/ACT half-chunk (16KB/partition loads)

_CACHE = {}


def _build(debug=False, variant=""):
    from contextlib import ExitStack

    import concourse.bacc as bacc
    import concourse.bass as bass
    import concourse.mybir as mybir
    import concourse.tile as tile
    from concourse.masks import make_identity

    f32 = mybir.dt.float32
    bf16 = mybir.dt.bfloat16
    u32 = mybir.dt.uint32
    AF = mybir.ActivationFunctionType
    OP = mybir.AluOpType
    AX = mybir.AxisListType

    nc = bacc.Bacc()
    nodes_h = nc.declare_dram_parameter("nodes", [BLOC, 2, 2], f32, isOutput=False)
    ancS_h = nc.declare_dram_parameter("ancS", [BLOC, M, 2], f32, isOutput=False)
    ancL_h = nc.declare_dram_parameter("ancL", [BLOC, M, 2], f32, isOutput=False)
    W1_h = nc.declare_dram_parameter("W1", [EMB, 2], f32, isOutput=False)
    b1_h = nc.declare_dram_parameter("b1", [EMB], f32, isOutput=False)
    W2_h = nc.declare_dram_parameter("W2", [EMB, EMB], f32, isOutput=False)
    b2_h = nc.declare_dram_parameter("b2", [EMB], f32, isOutput=False)
    out_h = nc.declare_dram_parameter("out", [BLOC, 2 * EMB], f32, isOutput=True)
    if debug:
        dbgf_h = nc.declare_dram_parameter("dbgf", [BLOC, 2, 32], f32, isOutput=True)
        dbgi_h = nc.declare_dram_parameter("dbgi", [BLOC, 2, 16], u32, isOutput=True)

    with ExitStack() as ctx:
        tc = ctx.enter_context(tile.TileContext(nc))
        const = ctx.enter_context(tc.tile_pool(name="const", bufs=1))
        a_pool = ctx.enter_context(tc.tile_pool(name="a", bufs=8))
        sq_pool = ctx.enter_context(tc.tile_pool(name="sq", bufs=2))
        d2_pool = ctx.enter_context(tc.tile_pool(name="d2", bufs=1))
        fp1 = ctx.enter_context(tc.tile_pool(name="fp1", bufs=1))
        fp2 = ctx.enter_context(tc.tile_pool(name="fp2", bufs=1))
        slot_pool = ctx.enter_context(tc.tile_pool(name="slot", bufs=2))
        cand_pool = ctx.enter_context(tc.tile_pool(name="cand", bufs=4))
        sel_pool = ctx.enter_context(tc.tile_pool(name="sel", bufs=4))
        cwork = ctx.enter_context(tc.tile_pool(name="cwork", bufs=2))
        cw2 = ctx.enter_context(tc.tile_pool(name="cw2", bufs=1))
        small = ctx.enter_context(tc.tile_pool(name="small", bufs=2))
        mlp = ctx.enter_context(tc.tile_pool(name="mlp", bufs=1))
        psum_tp = ctx.enter_context(tc.tile_pool(name="psum_tp", bufs=1, space="PSUM"))
        psum_mm = ctx.enter_context(tc.tile_pool(name="psum_mm", bufs=1, space="PSUM"))
        psum_h2 = ctx.enter_context(tc.tile_pool(name="psum_h2", bufs=1, space="PSUM"))

        # nodes first, on gpsimd: negn gates every front ACT, so it must not
        # queue behind other const traffic
        nodes_all = const.tile([P, 4 * NBT], f32)
        nc.gpsimd.dma_start(
            out=nodes_all[:].rearrange("p (t x) -> p t x", x=4),
            in_=nodes_h[:].rearrange("(t p) a c -> p t (a c)", p=P),
        )
        negn_all = const.tile([P, 4 * NBT], f32)
        nc.gpsimd.tensor_scalar(
            out=negn_all[:], in0=nodes_all[:], scalar1=-1.0, scalar2=None, op0=OP.mult
        )

        ident = const.tile([P, P], f32)
        make_identity(nc, ident[:])

        # Warm-up Gelu: anchors the ACT table chooser on gelu_and_others
        # (gelu/square/tanh/copy) so the kernel needs a single table load.
        dummy = const.tile([1, 1], f32)
        nc.vector.memset(dummy[:], 0.0)
        nc.scalar.activation(dummy[:], dummy[:], AF.Gelu, bias=0.0, scale=1.0)

        # W1/W2 loaded straight (contiguous rows) and transposed on the tensor
        # engine — a transposing DMA of W2 costs ~35us of descriptor work
        w1sb = const.tile([EMB, 2], f32)
        nc.scalar.dma_start(out=w1sb[:], in_=W1_h[:])
        w2sb = const.tile([EMB, EMB], f32)
        nc.scalar.dma_start(out=w2sb[:], in_=W2_h[:])
        b1c = const.tile([EMB, 1], f32)
        nc.scalar.dma_start(out=b1c[:], in_=b1_h[:].rearrange("(e u) -> e u", u=1))
        b2c = const.tile([EMB, 1], f32)
        nc.scalar.dma_start(out=b2c[:], in_=b2_h[:].rearrange("(e u) -> e u", u=1))
        w1ps = psum_mm.tile([2, EMB], f32, tag="hp")
        nc.tensor.transpose(out=w1ps[:], in_=w1sb[:], identity=ident[0:EMB, 0:EMB])
        w1t = const.tile([2, EMB], f32)  # w1t[c, e] = W1[e, c]
        nc.vector.tensor_copy(w1t[:], w1ps[:])
        w2ps = psum_mm.tile([EMB, EMB], f32, tag="h2p")
        nc.tensor.transpose(out=w2ps[:], in_=w2sb[:], identity=ident[0:EMB, 0:EMB])
        w2t = const.tile([EMB, EMB], f32)  # w2t[e, f] = W2[f, e]
        nc.vector.tensor_copy(w2t[:], w2ps[:])

        # block-diag MLP weights: one matmul handles a pair of neighbors.
        # w1blk[(k c), (k' e)] = W1[e, c] * delta(k, k'); the same tile serves
        # pairs (0,1) and (2,3) since the blocks repeat.
        w1blk = const.tile([4, 2 * EMB], f32)
        nc.vector.memset(w1blk[:], 0.0)
        nc.vector.tensor_copy(w1blk[0:2, 0:EMB], w1t[:])
        # compute engines cannot address a partition base of 2; SBUF->SBUF DMA
        # has no such restriction
        nc.gpsimd.dma_start(out=w1blk[2:4, EMB:2 * EMB], in_=w1t[:])
        w2blk = const.tile([2 * EMB, 2 * EMB], f32)
        nc.vector.memset(w2blk[:], 0.0)
        nc.vector.tensor_copy(w2blk[0:EMB, 0:EMB], w2t[:])
        nc.vector.tensor_copy(w2blk[EMB:2 * EMB, EMB:2 * EMB], w2t[:])
        b1blk = const.tile([2 * EMB, 1], f32)
        nc.vector.tensor_copy(b1blk[0:EMB, :], b1c[:])
        nc.vector.tensor_copy(b1blk[EMB:2 * EMB, :], b1c[:])
        b2blk = const.tile([2 * EMB, 1], f32)
        nc.vector.tensor_copy(b2blk[0:EMB, :], b2c[:])
        nc.vector.tensor_copy(b2blk[EMB:2 * EMB, :], b2c[:])

        # iota2f[p, j] = j // 2 as f32: [0,0,1,1,...] — onehot domain over the
        # interleaved (m c) candidate layout
        iotau = cw2.tile([P, 2 * NCAND], u32, tag="oh")
        nc.gpsimd.iota(iotau[:], pattern=[[1, 2 * NCAND]], base=0, channel_multiplier=0)
        iota2u = cw2.tile([P, 2 * NCAND], u32, tag="prod")
        nc.vector.tensor_scalar(
            out=iota2u[:], in0=iotau[:], scalar1=1, scalar2=None,
            op0=OP.logical_shift_right,
        )
        iota2f = const.tile([P, 2 * NCAND], f32)
        nc.vector.tensor_copy(iota2f[:], iota2u[:])

        # rowbase_all[p, t] = (t*P + p) * NSLOT (offsets into the (b s) axis);
        # iota steps must fit int16, so build t*P + p then shift
        rowtmp = const.tile([P, NBT], u32)
        nc.gpsimd.iota(rowtmp[:], pattern=[[P, NBT]], base=0, channel_multiplier=1)
        rowbase_all = const.tile([P, NBT], u32)
        nc.vector.tensor_scalar(
            out=rowbase_all[:], in0=rowtmp[:], scalar1=9, scalar2=None,
            op0=OP.logical_shift_left,
        )

        # flat view of anchors for the slot rescan: row r = b*NSLOT + s holds
        # the 16 (x, y) pairs of slot s of batch-row b (32 f32 = 128B)
        ancS_slots = ancS_h[:].rearrange("b (s j) c -> (b s) (j c)", j=SLOT)
        ancL_slots = ancL_h[:].rearrange("b (s j) c -> (b s) (j c)", j=SLOT)

        def front(bt, br):
            """distance sweep + slot top-k + rescan gather issue"""
            rows = slice(bt * P, (bt + 1) * P)
            anc_h = ancS_h if br == 0 else ancL_h
            anc_slots = ancS_slots if br == 0 else ancL_slots
            nbx = negn_all[:, 4 * bt + 2 * br:4 * bt + 2 * br + 1]
            nby = negn_all[:, 4 * bt + 2 * br + 1:4 * bt + 2 * br + 2]

            nslot = slot_pool.tile([P, NSLOT], f32)  # -min(d2) per slot
            f2_c0 = None
            for chk in range(NCH):
                # loads, squares and pair-adds run at half-chunk
                # granularity: 8 ring slots of 16KB/partition keep the sync
                # HWDGE ring far enough ahead that ACTs never wait on DMA
                d2c = d2_pool.tile([P, CM], bf16)
                for h in range(2):
                    a_t = a_pool.tile([P, 2 * HC], f32)
                    lo = chk * CM + h * HC
                    nc.sync.dma_start(
                        out=a_t[:],
                        in_=anc_h[rows, lo:lo + HC, :].rearrange("p m c -> p (m c)"),
                    )
                    av = a_t[:].rearrange("p (m c) -> p m c", c=2)
                    u2 = sq_pool.tile([P, HC], bf16)
                    v2 = sq_pool.tile([P, HC], bf16)
                    nc.scalar.activation(
                        u2[:], av[:, :, 0], AF.Square, bias=nbx, scale=1.0
                    )
                    nc.scalar.activation(
                        v2[:], av[:, :, 1], AF.Square, bias=nby, scale=1.0
                    )
                    nc.vector.tensor_tensor(
                        out=d2c[:, h * HC:(h + 1) * HC],
                        in0=u2[:], in1=v2[:], op=OP.add,
                    )
                f1 = fp1.tile([P, CM // 2], bf16)
                dv = d2c[:].rearrange("p (s two j) -> p s two j", two=2, j=8)
                nc.vector.tensor_tensor(
                    out=f1[:].rearrange("p (s j) -> p s j", j=8),
                    in0=dv[:, :, 0, :], in1=dv[:, :, 1, :], op=OP.min,
                )
                f2 = fp2.tile([P, CM // 4], bf16)
                fv1 = f1[:].rearrange("p (s two j) -> p s two j", two=2, j=4)
                nc.vector.tensor_tensor(
                    out=f2[:].rearrange("p (s j) -> p s j", j=4),
                    in0=fv1[:, :, 0, :], in1=fv1[:, :, 1, :], op=OP.min,
                )
                nc.vector.tensor_reduce(
                    out=nslot[:, chk * NSC:(chk + 1) * NSC],
                    in_=f2[:].rearrange("p (s j) -> p s j", j=4),
                    axis=AX.X, op=OP.min, negate=True,
                )
                if chk == 0:
                    f2_c0 = f2

            svals8 = sel_pool.tile([P, 8], f32)
            nc.vector.max(out=svals8[:], in_=nslot[:])
            sidx8 = sel_pool.tile([P, 8], u32)
            nc.vector.max_index(out=sidx8[:], in_max=svals8[:], in_values=nslot[:])

            # offsets into the (b s) axis of the slot-flattened anchors
            soff = sel_pool.tile([P, 8], u32)
            nc.vector.tensor_tensor(
                out=soff[:], in0=sidx8[:],
                in1=rowbase_all[:, bt:bt + 1].to_broadcast([P, 8]), op=OP.add,
            )

            # rescan: pull the top slots' raw coords (16 pairs each).
            # HW DGE honors one offset per partition per indirect DMA, so one
            # DMA per slot rank.
            cand = cand_pool.tile([P, 2 * NCAND], f32)
            for r in range(NRESC):
                nc.gpsimd.indirect_dma_start(
                    out=cand[:, 32 * r:32 * (r + 1)],
                    out_offset=None,
                    in_=anc_slots,
                    in_offset=bass.IndirectOffsetOnAxis(
                        ap=soff[:, r:r + 1], axis=0
                    ),
                )
            return {"cand": cand, "nbx": nbx, "nby": nby, "bt": bt, "br": br,
                    "svals8": svals8, "sidx8": sidx8, "nslot": nslot,
                    "f2c0": f2_c0}

        def tail(st, gate=None):
            """exact rescan, top-4 select, coord extraction, softmax, MLP"""
            bt, br, cand = st["bt"], st["br"], st["cand"]
            rows = slice(bt * P, (bt + 1) * P)
            cv = cand[:].rearrange("p (m c) -> p m c", c=2)

            # The Tile scheduler orders by its own cost model, which assumes
            # gather DMAs land instantly — it would place these ACTs right
            # after this unit's gathers and stall scalar ~10us per unit.
            # Routing the bias through an op that also reads a LATER front's
            # nslot forces the rescan 2+ units behind its gathers.
            nbx, nby = st["nbx"], st["nby"]
            if gate is not None:
                bt_, br_ = st["bt"], st["br"]
                gat = small.tile([P, 2], f32, tag="gat")
                nc.vector.scalar_tensor_tensor(
                    out=gat[:], in0=gate[:, 0:1].to_broadcast([P, 2]), scalar=0.0,
                    in1=negn_all[:, 4 * bt_ + 2 * br_:4 * bt_ + 2 * br_ + 2],
                    op0=OP.mult, op1=OP.add,
                )
                nbx, nby = gat[:, 0:1], gat[:, 1:2]

            cu2 = cwork.tile([P, NCAND], f32)
            cv2 = cwork.tile([P, NCAND], f32)
            nc.scalar.activation(
                cu2[:], cv[:, :, 0], AF.Square, bias=nbx, scale=1.0
            )
            nc.scalar.activation(
                cv2[:], cv[:, :, 1], AF.Square, bias=nby, scale=1.0
            )
            ncd = cwork.tile([P, NCAND], f32)  # exact -d2 of candidates
            nc.vector.scalar_tensor_tensor(
                out=ncd[:], in0=cu2[:], scalar=-1.0, in1=cv2[:],
                op0=OP.mult, op1=OP.subtract,
            )

            cvals8 = small.tile([P, 8], f32)
            nc.vector.max(out=cvals8[:], in_=ncd[:])
            cpos8 = small.tile([P, 8], u32)
            nc.vector.max_index(out=cpos8[:], in_max=cvals8[:], in_values=ncd[:])
            cposf = small.tile([P, K], f32)
            nc.vector.tensor_copy(cposf[:], cpos8[:, 0:K])

            # extract the top-4 coords straight from the interleaved cand in
            # three wide ops: onehot(k x cand) -> mask -> strided pair reduce
            oh = cw2.tile([P, K * 2 * NCAND], bf16, tag="oh")
            ohv = oh[:].rearrange("p (k j) -> p k j", k=K)
            nc.vector.tensor_tensor(
                out=ohv,
                in0=iota2f[:].rearrange("p (u j) -> p u j", u=1).to_broadcast(
                    [P, K, 2 * NCAND]
                ),
                in1=cposf[:].rearrange("p (k u) -> p k u", u=1).to_broadcast(
                    [P, K, 2 * NCAND]
                ),
                op=OP.is_equal,
            )
            prod = cw2.tile([P, K * 2 * NCAND], f32, tag="prod")
            nc.gpsimd.tensor_tensor(
                out=prod[:].rearrange("p (k j) -> p k j", k=K),
                in0=ohv,
                in1=cand[:].rearrange("p (u j) -> p u j", u=1).to_broadcast(
                    [P, K, 2 * NCAND]
                ),
                op=OP.mult,
            )
            # comb[p, 2k + c] = sum_m prod[p, k, m, c]
            comb = small.tile([P, 2 * K], f32)
            nc.vector.tensor_reduce(
                out=comb[:].rearrange("p (k c) -> p k c", c=2),
                in_=prod[:].rearrange("p (k m c) -> p k c m", c=2, m=NCAND),
                axis=AX.X, op=OP.add,
            )

            # w = softmax(d2_topk / tau); cvals8[:, :K] = -d2 (descending),
            # so the softmax argmax is at column K-1.
            nrmaxh = small.tile([P, 1], f32)
            nc.vector.tensor_scalar(
                out=nrmaxh[:], in0=cvals8[:, K - 1:K], scalar1=float(0.5 / TAU),
                scalar2=None, op0=OP.mult,
            )
            # exp(x) = 2/(1 - tanh(x/2)) - 1; keeps ACT inside the
            # gelu_and_others table set (no exp there).
            th = small.tile([P, K], f32)
            nc.scalar.activation(
                th[:], cvals8[:, 0:K], AF.Tanh,
                bias=nrmaxh[:, 0:1], scale=float(-0.5 / TAU),
            )
            denom = small.tile([P, K], f32)
            nc.vector.tensor_scalar(
                out=denom[:], in0=th[:], scalar1=-1.0, scalar2=1.0,
                op0=OP.mult, op1=OP.add,
            )
            rden = small.tile([P, K], f32)
            nc.vector.reciprocal(rden[:], denom[:])
            e4 = small.tile([P, K], f32)
            nc.vector.tensor_scalar(
                out=e4[:], in0=rden[:], scalar1=2.0, scalar2=-1.0,
                op0=OP.mult, op1=OP.add,
            )
            ssum = small.tile([P, 1], f32)
            nc.vector.reduce_sum(out=ssum[:], in_=e4[:], axis=AX.X)
            rinv = small.tile([P, 1], f32)
            nc.vector.reciprocal(rinv[:], ssum[:])
            wnorm = small.tile([P, K], f32)
            nc.vector.tensor_scalar(
                out=wnorm[:], in0=e4[:], scalar1=rinv[:, 0:1],
                scalar2=None, op0=OP.mult,
            )

            # comb [P, (k c)] -> combT [(k c), P] in one transpose; the
            # block-diag weights then run the MLP for a k-pair per matmul
            # each k-pair's coords transposed into its own base-0 tile
            # (partition bases must be 0/32/64 for compute-engine access)
            ct_ps_a = psum_tp.tile([4, P], f32, tag="ct_a")
            nc.tensor.transpose(out=ct_ps_a[:], in_=comb[:, 0:4], identity=ident[:])
            ct_ps_b = psum_tp.tile([4, P], f32, tag="ct_b")
            nc.tensor.transpose(out=ct_ps_b[:], in_=comb[:, 4:8], identity=ident[:])
            # both k-pairs side by side: one matmul and one gelu per layer
            combT = mlp.tile([4, 2 * P], f32)
            nc.vector.tensor_copy(combT[:, 0:P], ct_ps_a[:])
            nc.vector.tensor_copy(combT[:, P:2 * P], ct_ps_b[:])

            hp = psum_mm.tile([2 * EMB, 2 * P], f32, tag="hp")
            nc.tensor.matmul(
                out=hp[:], lhsT=w1blk[:], rhs=combT[:], start=True, stop=True
            )
            h1 = mlp.tile([2 * EMB, 2 * P], f32)
            nc.scalar.activation(
                h1[:], hp[:], AF.Gelu, bias=b1blk[:, 0:1], scale=1.0
            )
            h2p = psum_mm.tile([2 * EMB, 2 * P], f32, tag="h2p")
            nc.tensor.matmul(
                out=h2p[:], lhsT=w2blk[:], rhs=h1[:], start=True, stop=True
            )
            h2 = mlp.tile([2 * EMB, 2 * P], f32)
            nc.scalar.activation(
                h2[:], h2p[:], AF.Gelu, bias=b2blk[:, 0:1], scale=1.0
            )

            # weighted sum over neighbors, back in b-on-partitions layout
            osb = small.tile([P, EMB], f32)
            h2t_a = psum_h2.tile([P, 2 * EMB], f32, tag="h2t_a")
            nc.tensor.transpose(out=h2t_a[:], in_=h2[:, 0:P], identity=ident[:])
            h2t_b = psum_h2.tile([P, 2 * EMB], f32, tag="h2t_b")
            nc.tensor.transpose(out=h2t_b[:], in_=h2[:, P:2 * P], identity=ident[:])
            nc.vector.tensor_scalar(
                out=osb[:], in0=h2t_a[:, 0:EMB], scalar1=wnorm[:, 0:1],
                scalar2=None, op0=OP.mult,
            )
            for k in range(1, K):
                srct = h2t_a if k < 2 else h2t_b
                col = (k % 2) * EMB
                nc.vector.scalar_tensor_tensor(
                    out=osb[:], in0=srct[:, col:col + EMB],
                    scalar=wnorm[:, k:k + 1],
                    in1=osb[:], op0=OP.mult, op1=OP.add,
                )
            # out store deferred to the next tail (issued from flush_out):
            # issuing it here would park gpsimd on the osb semaphore and
            # head-of-line block the next unit's gather DMAs
            st["osb"] = osb
            if debug:
                dbf = small.tile([P, 32], f32)
                nc.vector.tensor_copy(dbf[:, 0:8], cvals8[:])
                nc.vector.tensor_copy(dbf[:, 8:16], comb[:])
                nc.vector.tensor_copy(dbf[:, 16:20], wnorm[:])
                nc.vector.tensor_copy(dbf[:, 20:28], st["svals8"][:])
                nc.gpsimd.dma_start(out=dbgf_h[rows, br, :], in_=dbf[:])
                dbi = small.tile([P, 16], u32)
                nc.vector.tensor_copy(dbi[:, 0:8], st["sidx8"][:])
                nc.vector.tensor_copy(dbi[:, 8:16], cpos8[:])
                nc.gpsimd.dma_start(out=dbgi_h[rows, br, :], in_=dbi[:])

        def flush_out(st):
            """issue the (now long-ready) out store for an already-tailed unit"""
            rows = slice(st["bt"] * P, (st["bt"] + 1) * P)
            nc.gpsimd.dma_start(
                out=out_h[rows, st["br"] * EMB:(st["br"] + 1) * EMB],
                in_=st["osb"][:],
            )

        # software pipeline, 3 units deep: tail(i) is issued after front(i+3),
        # so a unit's select->gather->rescan chain (including the ~8us gather
        # completion latency) is hidden; out stores trail one more tail behind
        pending = []
        done = []
        for bt in range(NBT):
            for br in range(2):
                pending.append(front(bt, br))
                if len(pending) > 3:
                    st = pending.pop(0)
                    tail(st, gate=pending[1]["nslot"][:])
                    done.append(st)
                    if len(done) > 1:
                        flush_out(done.pop(0))
        while pending:
            st = pending.pop(0)
            tail(st)
            done.append(st)
        for st in done:
            flush_out(st)
    return nc


def _get_nc(debug=False, variant=""):
    key = ("nc", debug, variant)
    if key not in _CACHE:
        nc = _build(debug, variant)
        nc.finalize()  # runs the Bacc passes (event sems, reg alloc, table loads)
        _CACHE[key] = nc
    return _CACHE[key]


def _make_in_maps(inputs):
    nodes = np.asarray(inputs["nodes_2x2"], dtype=np.float32)
    ancS = np.asarray(inputs["ancS"], dtype=np.float32)
    ancL = np.asarray(inputs["ancL"], dtype=np.float32)
    W1 = np.asarray(inputs["W1"], dtype=np.float32)
    b1 = np.asarray(inputs["b1"], dtype=np.float32)
    W2 = np.asarray(inputs["W2"], dtype=np.float32)
    b2 = np.asarray(inputs["b2"], dtype=np.float32)
    in_maps = []
    for c in range(NCORES):
        sl = slice(c * BLOC, (c + 1) * BLOC)
        in_maps.append(
            {
                "nodes": np.ascontiguousarray(nodes[sl]),
                "ancS": np.ascontiguousarray(ancS[sl]),
                "ancL": np.ascontiguousarray(ancL[sl]),
                "W1": W1,
                "b1": b1,
                "W2": W2,
                "b2": b2,
            }
        )
    return in_maps


def _run(in_maps, trace=False, debug=False, variant=""):
    from concourse.bass_utils import run_bass_kernel_spmd

    nc = _get_nc(debug, variant)
    return run_bass_kernel_spmd(nc, in_maps, core_ids=list(range(NCORES)), trace=trace)


def kernel(**inputs):
    in_maps = _make_in_maps(inputs)
    res = _run(in_maps).results
    out = np.concatenate([res[c]["out"] for c in range(NCORES)], axis=0)
    return out[:, :EMB].copy(), out[:, EMB:].copy()
